# revision 7
# baseline (speedup 1.0000x reference)
"""Trainium2 Bass kernel for nn_AlignModel.

Computes out[b, j, i] = sigmoid(simp[b,j]·w_s + orig[b,i]·w_o + bias) where
orig/simp are the two halves of prop_state[b] ([B, 2S, D] -> [B,S,D] each),
w_o = W[0,:D], w_s = W[0,D:].

Sharding: data-parallel over batch B=8 across the 8 NeuronCores. Each core:
  in  x   [4096, 512] f32  (= prop_state[b])
  in  w   [1, 1024]   f32
  in  bvec[1, 1]      f32
  out out [2048, 2048] f32 (= sigmoid(s_s[:,None] + (s_o + b)[None,:]))

Layout trick: the orig half is consumed with i = p*16 + n (partition-outer)
so that (a) each 1 MiB input chunk is one contiguous 8 KiB DMA descriptor
per partition and (b) the s_o column matrix [128,16] flattens to the
broadcast row [1,2048] with a plain partition-major DMA — no transpose.

Per-core pipeline (engine assignment tuned from NTFF profiles):
  - Sync HWDGE queue carries inputs then the paired 2 MiB output stores,
    back-to-back, so the DMA pipe never idles.
  - w and b are replicated across partitions with zero-stride DMAs (SWDGE).
  - s_o: DVE tensor_mul + ScalarE Copy-with-accum (tiles 0-11) / DVE
    tensor_reduce (last 4, so the broadcast chain is not gated on ScalarE).
  - s_o [128,16] -> [1,2048] flatten (Scalar HWDGE queue), then 4 rank-1
    PE matmuls broadcast it into PSUM [128,2048].
  - s_s: DVE tensor_mul + tensor_reduce, +b folded per group.
  - per output row-tile t: ONE ScalarE op
      out_t = Sigmoid(s_o_bcast + bias_col_t)   (PSUM -> SBUF)
    pairs of row-tiles go out as single 2 MiB DMAs on the Sync queue.
"""

import numpy as np

import concourse.mybir as mybir
from concourse import bacc, bass_utils
from concourse.tile import TileContext

P = 128          # partitions
D = 512          # feature dim
S = 2048         # sents
NT = S // P      # 16 tiles per half
GROUP = 4        # tiles per input DMA (1 MiB chunks)
NG = NT // GROUP
NCORES = 8
F32 = mybir.dt.float32


def _kernel_body(tc, out, x, w, bvec):
    nc = tc.nc
    # orig half, partition-outer: i = p*NT + n
    xo_re = x[0:S, :].rearrange("(p n) d -> p n d", n=NT)
    # simp half, partition-inner: j = n*P + p  (bias needs column layout)
    xs_re = x[S:2 * S, :].rearrange("(n p) d -> p n d", p=P)

    with (
        tc.tile_pool(name="consts", bufs=1) as cpool,
        tc.tile_pool(name="xin", bufs=1) as xpool,
        tc.tile_pool(name="scratch", bufs=4) as spool,
        tc.tile_pool(name="outbuf", bufs=3) as opool,
        tc.tile_pool(name="psum", bufs=1, space="PSUM") as ppool,
    ):
        # --- input DMAs first, alone on the Sync HWDGE queue ---
        xo_tiles = []
        for c in range(NG):
            xo = xpool.tile([P, GROUP, D], F32, tag=f"xo{c}", name=f"xo{c}")
            nc.sync.dma_start(out=xo,
                              in_=xo_re[:, c * GROUP:(c + 1) * GROUP, :])
            xo_tiles.append(xo)
        xs_tiles = []
        for c in range(NG):
            xs = xpool.tile([P, GROUP, D], F32, tag=f"xs{c}", name=f"xs{c}")
            nc.sync.dma_start(out=xs,
                              in_=xs_re[:, c * GROUP:(c + 1) * GROUP, :])
            xs_tiles.append(xs)

        # w and b replicated across partitions by zero-stride DMA (SWDGE)
        w_bc = cpool.tile([P, 2 * D], F32, tag="wbc")
        nc.gpsimd.dma_start(out=w_bc, in_=w.broadcast_to([P, 2 * D]))
        b_col = cpool.tile([P, 1], F32, tag="bcol")
        nc.gpsimd.dma_start(out=b_col, in_=bvec.broadcast_to([P, 1]))
        ones_row = cpool.tile([1, P], F32, tag="ones")
        nc.gpsimd.memset(ones_row, 1.0)

        s_o_mat = cpool.tile([P, NT], F32, tag="somat")   # s_o[p*16+n] @ [p,n]
        s_sb_mat = cpool.tile([P, NT], F32, tag="ssmat")  # s_s + b, col t
        sob_psum = ppool.tile([P, S], F32, tag="sob")     # broadcast s_o rows

        # --- phase 1a: orig half -> s_o ---
        for c in range(NG):
            xo = xo_tiles[c]
            for blk in range(GROUP):
                t = c * GROUP + blk
                prod = spool.tile([P, D], F32, tag="prod", name=f"po{t}")
                nc.vector.tensor_mul(out=prod, in0=xo[:, blk, :],
                                     in1=w_bc[:, 0:D])
                if t < NT - 4:
                    nc.scalar.activation(
                        prod, prod, mybir.ActivationFunctionType.Copy,
                        accum_out=s_o_mat[:, t:t + 1])
                else:
                    nc.vector.tensor_reduce(
                        s_o_mat[:, t:t + 1], prod,
                        axis=mybir.AxisListType.X, op=mybir.AluOpType.add)

        # --- flatten s_o to a row (partition-major) and broadcast via PE ---
        so_row = cpool.tile([1, S], F32, tag="sorow")
        nc.scalar.dma_start(out=so_row, in_=s_o_mat)   # [128,16] -> [1,2048]
        for j in range(S // 512):
            nc.tensor.matmul(sob_psum[:, j * 512:(j + 1) * 512], ones_row,
                             so_row[:, j * 512:(j + 1) * 512],
                             start=True, stop=True)

        # --- phase 1b + 2: simp half -> s_s + b, then outputs ---
        o_sb = None
        for g in range(NG):
            xs = xs_tiles[g]
            for blk in range(GROUP):
                t = g * GROUP + blk
                prod = spool.tile([P, D], F32, tag="prod", name=f"ps{t}")
                nc.vector.tensor_mul(out=prod, in0=xs[:, blk, :],
                                     in1=w_bc[:, D:2 * D])
                nc.vector.tensor_reduce(
                    s_sb_mat[:, t:t + 1], prod,
                    axis=mybir.AxisListType.X, op=mybir.AluOpType.add)
            nc.vector.tensor_scalar_add(
                s_sb_mat[:, g * GROUP:(g + 1) * GROUP],
                s_sb_mat[:, g * GROUP:(g + 1) * GROUP], b_col)
            for blk in range(GROUP):
                t = g * GROUP + blk
                q = t % 2
                if q == 0:
                    o_sb = opool.tile([P, 2, S], F32, tag="osb",
                                      name=f"opair{t // 2}")
                nc.scalar.activation(
                    o_sb[:, q, :], sob_psum,
                    mybir.ActivationFunctionType.Sigmoid,
                    bias=s_sb_mat[:, t:t + 1],
                    scale=1.0,
                )
                if q == 1:
                    r0 = (t - 1) * P
                    dst = out[r0:r0 + 2 * P, :].rearrange(
                        "(q p) i -> p q i", p=P)
                    nc.sync.dma_start(out=dst, in_=o_sb)


def build_program():
    nc = bacc.Bacc(
        "TRN2",
        debug=False,
        target_bir_lowering=False,
        num_devices=NCORES,
    )
    x = nc.dram_tensor("x", [2 * S, D], F32, kind="ExternalInput").ap()
    w = nc.dram_tensor("w", [1, 2 * D], F32, kind="ExternalInput").ap()
    bvec = nc.dram_tensor("bvec", [1, 1], F32, kind="ExternalInput").ap()
    out = nc.dram_tensor("out", [S, S], F32, kind="ExternalOutput").ap()
    with TileContext(nc) as tc:
        _kernel_body(tc, out, x, w, bvec)
    nc.compile()
    return nc


_PROGRAM = None


def _get_program():
    global _PROGRAM
    if _PROGRAM is None:
        _PROGRAM = build_program()
    return _PROGRAM


def make_in_maps(prop_state, W, b):
    prop = np.ascontiguousarray(np.asarray(prop_state, dtype=np.float32))
    w = np.ascontiguousarray(np.asarray(W, dtype=np.float32).reshape(1, 2 * D))
    bv = np.ascontiguousarray(np.asarray(b, dtype=np.float32).reshape(1, 1))
    assert prop.shape == (NCORES, 2 * S, D), prop.shape
    return [{"x": prop[i], "w": w, "bvec": bv} for i in range(NCORES)]


def kernel(A, prop_state, W, b, _trace=False):
    nc = _get_program()
    in_maps = make_in_maps(prop_state, W, b)
    res = bass_utils.run_bass_kernel_spmd(
        nc, in_maps, core_ids=list(range(NCORES)), trace=_trace)
    out = np.stack([res.results[i]["out"] for i in range(NCORES)], axis=0)
    if _trace:
        kernel.last_results = res
    return out


# revision 8
# speedup vs baseline: 1.0233x; 1.0233x over previous
"""Trainium2 Bass kernel for nn_AlignModel.

Computes out[b, j, i] = sigmoid(simp[b,j]·w_s + orig[b,i]·w_o + bias) where
orig/simp are the two halves of prop_state[b] ([B, 2S, D] -> [B,S,D] each),
w_o = W[0,:D], w_s = W[0,D:].

Sharding: data-parallel over batch B=8 across the 8 NeuronCores. Each core:
  in  x   [4096, 512] f32  (= prop_state[b])
  in  w   [1, 1024]   f32
  in  bvec[1, 1]      f32
  out out [2048, 2048] f32 (= sigmoid(s_s[:,None] + (s_o + b)[None,:]))

Layout trick: the orig half is consumed with i = p*16 + n (partition-outer)
so that (a) each 1 MiB input chunk is one contiguous 8 KiB DMA descriptor
per partition and (b) the s_o column matrix [128,16] flattens to the
broadcast row [1,2048] with a plain partition-major DMA — no transpose.

Per-core pipeline (engine assignment tuned from NTFF profiles):
  - Sync HWDGE queue carries inputs then the paired 2 MiB output stores,
    back-to-back, so the DMA pipe never idles.
  - w and b are replicated across partitions with zero-stride DMAs (SWDGE).
  - s_o: DVE tensor_mul + ScalarE Copy-with-accum (tiles 0-11) / DVE
    tensor_reduce (last 4, so the broadcast chain is not gated on ScalarE).
  - s_o [128,16] -> [1,2048] flatten (Scalar HWDGE queue), then 4 rank-1
    PE matmuls broadcast it into PSUM [128,2048].
  - s_s: DVE tensor_mul + tensor_reduce, +b folded per group.
  - per output row-tile t: ONE ScalarE op
      out_t = Sigmoid(s_o_bcast + bias_col_t)   (PSUM -> SBUF)
    pairs of row-tiles go out as single 2 MiB DMAs on the Sync queue.
"""

import numpy as np

import concourse.mybir as mybir
from concourse import bacc, bass_utils
from concourse.tile import TileContext

P = 128          # partitions
D = 512          # feature dim
S = 2048         # sents
NT = S // P      # 16 tiles per half
GROUP = 4        # tiles per input DMA (1 MiB chunks)
NG = NT // GROUP
NCORES = 8
F32 = mybir.dt.float32


def _kernel_body(tc, out, x, w, bvec):
    nc = tc.nc
    # orig half, partition-outer: i = p*NT + n
    xo_re = x[0:S, :].rearrange("(p n) d -> p n d", n=NT)
    # simp half, partition-inner: j = n*P + p  (bias needs column layout)
    xs_re = x[S:2 * S, :].rearrange("(n p) d -> p n d", p=P)

    with (
        tc.tile_pool(name="consts", bufs=1) as cpool,
        tc.tile_pool(name="xin", bufs=1) as xpool,
        tc.tile_pool(name="scratch", bufs=4) as spool,
        tc.tile_pool(name="outbuf", bufs=3) as opool,
        tc.tile_pool(name="psum", bufs=1, space="PSUM") as ppool,
    ):
        # --- input DMAs first, alone on the Sync HWDGE queue ---
        xo_tiles = []
        for c in range(NG):
            xo = xpool.tile([P, GROUP, D], F32, tag=f"xo{c}", name=f"xo{c}")
            nc.sync.dma_start(out=xo,
                              in_=xo_re[:, c * GROUP:(c + 1) * GROUP, :])
            xo_tiles.append(xo)
        xs_tiles = []
        for c in range(NG):
            xs = xpool.tile([P, GROUP, D], F32, tag=f"xs{c}", name=f"xs{c}")
            nc.sync.dma_start(out=xs,
                              in_=xs_re[:, c * GROUP:(c + 1) * GROUP, :])
            xs_tiles.append(xs)

        # w and b replicated across partitions by zero-stride DMA (SWDGE)
        w_bc = cpool.tile([P, 2 * D], F32, tag="wbc")
        nc.gpsimd.dma_start(out=w_bc, in_=w.broadcast_to([P, 2 * D]))
        b_col = cpool.tile([P, 1], F32, tag="bcol")
        nc.gpsimd.dma_start(out=b_col, in_=bvec.broadcast_to([P, 1]))
        ones_row = cpool.tile([1, P], F32, tag="ones")
        nc.gpsimd.memset(ones_row, 1.0)

        s_o_mat = cpool.tile([P, NT], F32, tag="somat")   # s_o[p*16+n] @ [p,n]
        s_sb_mat = cpool.tile([P, NT], F32, tag="ssmat")  # s_s + b, col t
        sob_psum = ppool.tile([P, S], F32, tag="sob")     # broadcast s_o rows
        junk_psum = ppool.tile([P, 512], F32, tag="junk")  # PE warm-up target
        so_row = cpool.tile([1, S], F32, tag="sorow")

        # --- phase 1a: orig half -> s_o ---
        # DVE mul + ScalarE Copy-with-accum reduce; after each chunk the 4
        # fresh s_o columns are scattered into so_row (i = p*16+n) by a
        # small strided DMA so no flatten remains on the tail.  A dummy
        # matmul per chunk keeps the PE HAM-warm for the broadcast matmuls.
        for c in range(NG):
            xo = xo_tiles[c]
            for blk in range(GROUP):
                t = c * GROUP + blk
                prod = spool.tile([P, D], F32, tag="prod", name=f"po{t}")
                nc.vector.tensor_mul(out=prod, in0=xo[:, blk, :],
                                     in1=w_bc[:, 0:D])
                nc.scalar.activation(
                    prod, prod, mybir.ActivationFunctionType.Copy,
                    accum_out=s_o_mat[:, t:t + 1])
            src = s_o_mat[:, c * GROUP:(c + 1) * GROUP]
            dst = so_row.rearrange("o (p n) -> o p n", n=NT)[
                :, :, c * GROUP:(c + 1) * GROUP]
            nc.scalar.dma_start(out=dst, in_=src)
            nc.tensor.matmul(junk_psum, ones_row, w_bc[0:1, 0:512],
                             start=True, stop=True)
            nc.tensor.matmul(junk_psum, ones_row, w_bc[0:1, 0:512],
                             start=True, stop=True)

        # --- broadcast s_o + b across partitions via rank-1 matmuls ---
        for j in range(S // 512):
            nc.tensor.matmul(sob_psum[:, j * 512:(j + 1) * 512], ones_row,
                             so_row[:, j * 512:(j + 1) * 512],
                             start=True, stop=True)

        # --- phase 1b + 2: simp half -> s_s + b, then outputs ---
        o_sb = None
        for g in range(NG):
            xs = xs_tiles[g]
            for blk in range(GROUP):
                t = g * GROUP + blk
                prod = spool.tile([P, D], F32, tag="prod", name=f"ps{t}")
                nc.vector.tensor_mul(out=prod, in0=xs[:, blk, :],
                                     in1=w_bc[:, D:2 * D])
                nc.vector.tensor_reduce(
                    s_sb_mat[:, t:t + 1], prod,
                    axis=mybir.AxisListType.X, op=mybir.AluOpType.add)
            nc.vector.tensor_scalar_add(
                s_sb_mat[:, g * GROUP:(g + 1) * GROUP],
                s_sb_mat[:, g * GROUP:(g + 1) * GROUP], b_col)
            for blk in range(GROUP):
                t = g * GROUP + blk
                q = t % 2
                if q == 0:
                    o_sb = opool.tile([P, 2, S], F32, tag="osb",
                                      name=f"opair{t // 2}")
                nc.scalar.activation(
                    o_sb[:, q, :], sob_psum,
                    mybir.ActivationFunctionType.Sigmoid,
                    bias=s_sb_mat[:, t:t + 1],
                    scale=1.0,
                )
                if q == 1:
                    r0 = (t - 1) * P
                    dst = out[r0:r0 + 2 * P, :].rearrange(
                        "(q p) i -> p q i", p=P)
                    nc.sync.dma_start(out=dst, in_=o_sb)


def build_program():
    nc = bacc.Bacc(
        "TRN2",
        debug=False,
        target_bir_lowering=False,
        num_devices=NCORES,
    )
    x = nc.dram_tensor("x", [2 * S, D], F32, kind="ExternalInput").ap()
    w = nc.dram_tensor("w", [1, 2 * D], F32, kind="ExternalInput").ap()
    bvec = nc.dram_tensor("bvec", [1, 1], F32, kind="ExternalInput").ap()
    out = nc.dram_tensor("out", [S, S], F32, kind="ExternalOutput").ap()
    with TileContext(nc) as tc:
        _kernel_body(tc, out, x, w, bvec)
    nc.compile()
    return nc


_PROGRAM = None


def _get_program():
    global _PROGRAM
    if _PROGRAM is None:
        _PROGRAM = build_program()
    return _PROGRAM


def make_in_maps(prop_state, W, b):
    prop = np.ascontiguousarray(np.asarray(prop_state, dtype=np.float32))
    w = np.ascontiguousarray(np.asarray(W, dtype=np.float32).reshape(1, 2 * D))
    bv = np.ascontiguousarray(np.asarray(b, dtype=np.float32).reshape(1, 1))
    assert prop.shape == (NCORES, 2 * S, D), prop.shape
    return [{"x": prop[i], "w": w, "bvec": bv} for i in range(NCORES)]


def kernel(A, prop_state, W, b, _trace=False):
    nc = _get_program()
    in_maps = make_in_maps(prop_state, W, b)
    res = bass_utils.run_bass_kernel_spmd(
        nc, in_maps, core_ids=list(range(NCORES)), trace=_trace)
    out = np.stack([res.results[i]["out"] for i in range(NCORES)], axis=0)
    if _trace:
        kernel.last_results = res
    return out


# revision 11
# speedup vs baseline: 1.0533x; 1.0292x over previous
"""Trainium2 Bass kernel for nn_AlignModel.

Computes out[b, j, i] = sigmoid(simp[b,j]·w_s + orig[b,i]·w_o + bias) where
orig/simp are the two halves of prop_state[b] ([B, 2S, D] -> [B,S,D] each),
w_o = W[0,:D], w_s = W[0,D:].

Sharding: data-parallel over batch B=8 across the 8 NeuronCores. Each core:
  in  x   [4096, 512] f32  (= prop_state[b])
  in  w   [1, 1024]   f32
  in  bvec[1, 1]      f32
  out out [2048, 2048] f32 (= sigmoid(s_s[:,None] + s_o[None,:] + b))

Key structural choices (from NTFF profile iterations):
  - The Sync HWDGE queue is a strict FIFO, so its order IS the schedule:
    orig chunks (gate everything) -> simp chunks -> paired output stores.
    The DMA pipe then runs continuously from first load to last store.
  - The orig half is consumed partition-outer (i = p*16 + n) so each 1 MiB
    chunk is one contiguous 8 KiB descriptor per partition (fast), and
    s_o[128,16] scatters into the broadcast row [1,2048] with small strided
    DMAs per chunk - no transpose, no PE.
  - s_o row -> [128,2048] replication is a single zero-stride DMA
    (128 x 8 KiB descriptors), much faster than cold fp32 matmuls.
  - Dot products: DVE tensor_mul + ScalarE Copy-with-accum (orig),
    DVE tensor_reduce (simp), keeping ScalarE free for phase-2 sigmoids.
  - Each output row-tile is ONE ScalarE op:
      out_t = Sigmoid(s_o_bcast + bias_col_t); pairs of row-tiles leave as
    single 2 MiB DMAs behind the inputs on the Sync queue.
"""

import numpy as np

import concourse.mybir as mybir
from concourse import bacc, bass_utils
from concourse.tile import TileContext

P = 128          # partitions
D = 512          # feature dim
S = 2048         # sents
NT = S // P      # 16 tiles per half
GROUP = 4        # tiles per input DMA (1 MiB chunks)
NG = NT // GROUP
NCORES = 8
F32 = mybir.dt.float32


def _kernel_body(tc, out, x, w, bvec):
    nc = tc.nc
    # orig half, partition-outer: i = p*NT + n
    xo_re = x[0:S, :].rearrange("(p n) d -> p n d", n=NT)
    # simp half, partition-inner: j = n*P + p  (bias needs column layout)
    xs_re = x[S:2 * S, :].rearrange("(n p) d -> p n d", p=P)

    with (
        tc.tile_pool(name="consts", bufs=1) as cpool,
        tc.tile_pool(name="xin", bufs=1) as xpool,
        tc.tile_pool(name="scratch", bufs=4) as spool,
        tc.tile_pool(name="outbuf", bufs=3) as opool,
        tc.tile_pool(name="dscr", bufs=1, space="DRAM") as dpool,
    ):
        # --- input DMAs: orig first (they gate the whole kernel), then simp;
        # outputs will queue behind these on the same Sync FIFO ---
        xo_tiles = []
        for c in range(NG):
            xo = xpool.tile([P, GROUP, D], F32, tag=f"xo{c}", name=f"xo{c}")
            nc.sync.dma_start(out=xo,
                              in_=xo_re[:, c * GROUP:(c + 1) * GROUP, :])
            xo_tiles.append(xo)
        xs_tiles = []
        for c in range(NG):
            xs = xpool.tile([P, GROUP, D], F32, tag=f"xs{c}", name=f"xs{c}")
            nc.sync.dma_start(out=xs,
                              in_=xs_re[:, c * GROUP:(c + 1) * GROUP, :])
            xs_tiles.append(xs)

        # w and b replicated across partitions by zero-stride DMA (SWDGE)
        w_bc = cpool.tile([P, 2 * D], F32, tag="wbc")
        nc.gpsimd.dma_start(out=w_bc, in_=w.broadcast_to([P, 2 * D]))
        b_col = cpool.tile([P, 1], F32, tag="bcol")
        nc.gpsimd.dma_start(out=b_col, in_=bvec.broadcast_to([P, 1]))

        s_o_mat = cpool.tile([P, NT], F32, tag="somat")   # s_o[p*16+n] @ [p,n]
        s_sb_mat = cpool.tile([P, NT], F32, tag="ssmat")  # s_s + b, col t
        so_dram = dpool.tile([1, S], F32, tag="sodram")   # flattened s_o (HBM)
        sob = cpool.tile([P, S], F32, tag="sob")          # s_o on every row

        # --- phase 1a: orig half -> s_o -> so_row (strided scatter/chunk) ---
        for c in range(NG):
            xo = xo_tiles[c]
            for blk in range(GROUP):
                t = c * GROUP + blk
                prod = spool.tile([P, D], F32, tag="prod", name=f"po{t}")
                nc.vector.tensor_mul(out=prod, in0=xo[:, blk, :],
                                     in1=w_bc[:, 0:D])
                nc.scalar.activation(
                    prod, prod, mybir.ActivationFunctionType.Copy,
                    accum_out=s_o_mat[:, t:t + 1])
            src = s_o_mat[:, c * GROUP:(c + 1) * GROUP]
            dst = so_dram.rearrange("o (p n) -> o p n", n=NT)[
                :, :, c * GROUP:(c + 1) * GROUP]
            nc.scalar.dma_start(out=dst, in_=src)

        # --- replicate s_o across all partitions with one zero-stride DMA
        # (SBUF APs forbid partition step 0, DRAM APs don't -> bounce) ---
        nc.scalar.dma_start(out=sob, in_=so_dram.broadcast_to([P, S]))

        # --- phase 1b + 2: simp half -> s_s + b, then outputs ---
        o_sb = None
        for g in range(NG):
            xs = xs_tiles[g]
            for blk in range(GROUP):
                t = g * GROUP + blk
                prod = spool.tile([P, D], F32, tag="prod", name=f"ps{t}")
                nc.vector.tensor_mul(out=prod, in0=xs[:, blk, :],
                                     in1=w_bc[:, D:2 * D])
                nc.vector.tensor_reduce(
                    s_sb_mat[:, t:t + 1], prod,
                    axis=mybir.AxisListType.X, op=mybir.AluOpType.add)
            nc.vector.tensor_scalar_add(
                s_sb_mat[:, g * GROUP:(g + 1) * GROUP],
                s_sb_mat[:, g * GROUP:(g + 1) * GROUP], b_col)
            for blk in range(GROUP):
                t = g * GROUP + blk
                q = t % 2
                if q == 0:
                    o_sb = opool.tile([P, 2, S], F32, tag="osb",
                                      name=f"opair{t // 2}")
                nc.scalar.activation(
                    o_sb[:, q, :], sob,
                    mybir.ActivationFunctionType.Sigmoid,
                    bias=s_sb_mat[:, t:t + 1],
                    scale=1.0,
                )
                if q == 1:
                    r0 = (t - 1) * P
                    dst = out[r0:r0 + 2 * P, :].rearrange(
                        "(q p) i -> p q i", p=P)
                    nc.sync.dma_start(out=dst, in_=o_sb)


def build_program():
    nc = bacc.Bacc(
        "TRN2",
        debug=False,
        target_bir_lowering=False,
        num_devices=NCORES,
    )
    x = nc.dram_tensor("x", [2 * S, D], F32, kind="ExternalInput").ap()
    w = nc.dram_tensor("w", [1, 2 * D], F32, kind="ExternalInput").ap()
    bvec = nc.dram_tensor("bvec", [1, 1], F32, kind="ExternalInput").ap()
    out = nc.dram_tensor("out", [S, S], F32, kind="ExternalOutput").ap()
    with TileContext(nc) as tc:
        _kernel_body(tc, out, x, w, bvec)
    nc.compile()
    return nc


_PROGRAM = None


def _get_program():
    global _PROGRAM
    if _PROGRAM is None:
        _PROGRAM = build_program()
    return _PROGRAM


def make_in_maps(prop_state, W, b):
    prop = np.ascontiguousarray(np.asarray(prop_state, dtype=np.float32))
    w = np.ascontiguousarray(np.asarray(W, dtype=np.float32).reshape(1, 2 * D))
    bv = np.ascontiguousarray(np.asarray(b, dtype=np.float32).reshape(1, 1))
    assert prop.shape == (NCORES, 2 * S, D), prop.shape
    return [{"x": prop[i], "w": w, "bvec": bv} for i in range(NCORES)]


def kernel(A, prop_state, W, b, _trace=False):
    nc = _get_program()
    in_maps = make_in_maps(prop_state, W, b)
    res = bass_utils.run_bass_kernel_spmd(
        nc, in_maps, core_ids=list(range(NCORES)), trace=_trace)
    out = np.stack([res.results[i]["out"] for i in range(NCORES)], axis=0)
    if _trace:
        kernel.last_results = res
    return out
